# revision 21
# baseline (speedup 1.0000x reference)
"""Trainium2 Bass kernel for nn_AttentionLayer (4x2048x768, d_k=128, d_v=768).

Sharding (sequence-parallel over keys, data-parallel over batch):
8 cores; core c handles batch b=c//2 with KEY half h=c%2. Each core computes
q for ALL 2048 queries but k/v only for its own 1024 keys, then produces the
partial (unnormalized) attention numerator plus the partial softmax row sum.

fp8 DoubleRow acceleration (2x PE throughput, 256-deep contraction/inst) for
the two dominant matmuls (v-projection and attn@V numerator), with two error
mitigations that keep rel err ~1.1e-2 (< 2e-2 gate):

  1. expm1 trick: the matmul uses P' = exp(s) - 1 quantized to fp8e4 instead
     of exp(s). Softmax weights here are ~1 +- 0.35, so |P'| << |P| and the
     fp8 quantization error shrinks ~3x. The dropped "1" contributes
     colsum_v[n] = sum_t v[t,n] to every query's numerator and T to every
     row sum; both are restored EXACTLY on the host:
         out = (colsum_v + sum_cores P'8@v8/32) / (2048 + sum_cores P'8@1)
     with colsum_v = (sum_t x_t) @ Wv computed in f64 (tiny: 768x768).
  2. The same colsum restore also cancels the common-mode (p-bar-weighted)
     component of the v-side fp8 quantization error.

Numerics per core:
  q/k proj + scores: bf16 (score accuracy dominates overall error).
  exp -> P_hi bf16 (scalar ACT engine, same cost as baseline), then
  gpsimd computes P'8 = fp8(P_hi - 1).
  v-proj: x8 fp8 x wv8 (=32*Wv in fp8) DoubleRow -> psum = 32*v ->
  v8 = fp8(32*v) (vector cast). The 32x pre-scale keeps Wv's tiny uniform
  (+-0.036) values out of fp8e4's subnormal range; host divides by 32.
  numerator: P'8 x v8 DoubleRow, t-chunk pairs packed in the two slots.
  bk is dropped (softmax-invariant); bq/bv handled as in the baseline.

DMA: inputs host-repacked partition-major into few fat-row DMAs on the
scalar + sync rings (~0.17MB/us each), ordered by first PE use with WAW
gates so early q/k loads aren't starved. Output partial [128, 16, 769]
bf16, stored in qc pairs alternating rings.
"""

import sys

sys.path.insert(0, "/opt/trn_rl_repo")

import numpy as np
import ml_dtypes

B, T, DIN, DK, DV = 4, 2048, 768, 128, 768
NCORES = 8
TOWN = 1024  # own keys per core
CH = DIN // 128  # 6 contraction chunks over d_in
TCH = TOWN // 128  # 8 own-key chunks
QCH = T // 128  # 16 query chunks (all queries)
SCALE = 1.0 / float(np.sqrt(DK))
VSCALE = 32.0  # fp8 pre-scale on Wv (power of 2; host divides out)

_CACHE = {}


def _build():
    from contextlib import ExitStack

    from concourse import bacc, mybir, tile

    f32 = mybir.dt.float32
    bf16 = mybir.dt.bfloat16
    fp8 = mybir.dt.float8e4
    DR = mybir.MatmulPerfMode.DoubleRow

    nc = bacc.Bacc("TRN2", target_bir_lowering=False, debug=False)

    x_own = nc.dram_tensor("x_own", [128, 3, 2 * TOWN], bf16, kind="ExternalInput").ap()
    x_oth = nc.dram_tensor("x_oth", [128, CH, TOWN], bf16, kind="ExternalInput").ap()
    wqk = nc.dram_tensor("wqk", [128, CH, 2 * DK], bf16, kind="ExternalInput").ap()
    wv8 = nc.dram_tensor("wv8", [128, CH, DV], fp8, kind="ExternalInput").ap()
    bq = nc.dram_tensor("bq", [DK, 1], f32, kind="ExternalInput").ap()
    out = nc.dram_tensor("out", [128, QCH, DV + 1], bf16, kind="ExternalOutput").ap()

    with tile.TileContext(nc) as tc, ExitStack() as ctx:
        consts = ctx.enter_context(tc.tile_pool(name="consts", bufs=1))
        persist = ctx.enter_context(tc.tile_pool(name="persist", bufs=1))
        wpool = ctx.enter_context(tc.tile_pool(name="wpool", bufs=1))
        xpool = ctx.enter_context(tc.tile_pool(name="xpool", bufs=1))
        ph_pool = ctx.enter_context(tc.tile_pool(name="ph", bufs=3))
        out_pool = ctx.enter_context(tc.tile_pool(name="out_pool", bufs=4))
        ps_pool = ctx.enter_context(tc.tile_pool(name="ps", bufs=3, space="PSUM"))
        sc_pool = ctx.enter_context(tc.tile_pool(name="sc", bufs=2, space="PSUM"))

        # PE p-state warm-up: the DMA queues cannot issue before ~+7us of
        # framework preamble and the first input lands ~+11us, so burn the
        # ~6us clock ramp on data-free matmuls during that window. The
        # memset is the program's first instruction so it leads its queue.
        warm = consts.tile([128, 640], bf16)
        nc.vector.memset(warm[:], 0.0)
        for _ in range(14):
            ps_w = sc_pool.tile([128, 512], f32, tag="sc")
            nc.tensor.matmul(
                ps_w[:], warm[:, 0:128], warm[:, 128:640], start=True, stop=True
            )

        bq_sb = consts.tile([DK, 1], f32)
        nc.gpsimd.dma_start(out=bq_sb[:], in_=bq)

        qT_sb = persist.tile([128, T], bf16)  # [dk, q] all queries, q̂+bq
        kT_sb = persist.tile([128, TOWN], bf16)  # [dk, t-own]
        v8_sb = persist.tile([128, TCH, DV + 2], fp8)  # [t-part, chunk, 32v|1|0]
        pT8_sb = persist.tile([128, TCH, T], fp8)  # [t-part, chunk, q] = exp-1

        nc.vector.memset(v8_sb[:, :, DV : DV + 1], 1.0)
        nc.vector.memset(v8_sb[:, :, DV + 1 : DV + 2], 0.0)

        # x-own pairs: pair p holds chunks 2p|2p+1 side by side (4KB rows)
        xo_sb = xpool.tile([128, 3, 2 * TOWN], bf16)
        xt_sb = xpool.tile([128, CH, TOWN], bf16)
        x8o_sb = xpool.tile([128, 3, 2, TOWN], fp8)
        wqk_sb = wpool.tile([128, CH, 2 * DK], bf16)
        wv8_sb = wpool.tile([128, CH, DV], fp8)

        def xo(c):  # own-x chunk c -> [128, TOWN] slice of the pair tile
            return xo_sb[:, c // 2, (c % 2) * TOWN : (c % 2 + 1) * TOWN]

        # Input DMAs: x pairs land in q/k consumption order (pair0 sync,
        # pair1 scalar, pair2 sync). Each ring's DMAs are serialized by WAW
        # gates (the rings round-robin over all queued DMAs, so an ungated
        # late load steals bandwidth from the critical early ones). The fp8
        # copy of own-x (for the v-proj DoubleRow) is cast on-chip by the
        # vector engine, which is idle during q/k.
        nc.scalar.dma_start(out=wqk_sb[:], in_=wqk)
        nc.sync.dma_start(out=xo_sb[:, 0, :], in_=x_own[:, 0, :])
        nc.vector.tensor_copy(xo_sb[:, 1, 0:1], wqk_sb[:, 0, 0:1])
        nc.scalar.dma_start(out=xo_sb[:, 1, :], in_=x_own[:, 1, :])
        nc.vector.tensor_copy(xo_sb[:, 2, 0:1], xo_sb[:, 0, 0:1])
        nc.sync.dma_start(out=xo_sb[:, 2, :], in_=x_own[:, 2, :])
        nc.vector.tensor_copy(wv8_sb[:, 0, 0:1], xo_sb[:, 0, 0:1])
        nc.gpsimd.dma_start(out=wv8_sb[:], in_=wv8)
        nc.vector.tensor_copy(xt_sb[:, 0, 0:1], xo_sb[:, 2, 0:1])
        nc.vector.tensor_copy(xt_sb[:, 3, 0:1], xo_sb[:, 1, 0:1])
        nc.sync.dma_start(out=xt_sb[:, 0:3, :], in_=x_oth[:, 0:3, :])
        nc.scalar.dma_start(out=xt_sb[:, 3:6, :], in_=x_oth[:, 3:6, :])
        # on-chip bf16 -> fp8 casts of own-x, one per d_in chunk
        for p in range(3):
            for s in range(2):
                nc.vector.tensor_copy(
                    x8o_sb[:, p, s, :], xo_sb[:, p, s * TOWN : (s + 1) * TOWN]
                )

        def emit_scores_t(t, qh):
            # scores^T for one own-key chunk -> P'8 = fp8(exp(scale*s) - 1).
            # Two 512-col exps pipeline from two psum banks into one 1024-col
            # ph tile; a single 1024-col vector op does the -1 + fp8 cast.
            ph = ph_pool.tile([128, 1024], bf16, tag="ph")
            for n0 in (0, 512):
                ps_s = sc_pool.tile([128, 512], f32, tag="sc")
                nc.tensor.matmul(
                    ps_s[:],
                    kT_sb[:, t * 128 : (t + 1) * 128],
                    qT_sb[:, qh * 1024 + n0 : qh * 1024 + n0 + 512],
                    start=True,
                    stop=True,
                )
                nc.scalar.activation(
                    ph[:, n0 : n0 + 512],
                    ps_s[:],
                    mybir.ActivationFunctionType.Exp,
                    scale=SCALE,
                )
            nc.vector.tensor_scalar(
                out=pT8_sb[:, t, qh * 1024 : qh * 1024 + 1024],
                in0=ph[:],
                scalar1=1.0,
                scalar2=None,
                op0=mybir.AluOpType.subtract,
            )

        def emit_v_t(t):
            # v-projection for one own-key chunk: fp8 DoubleRow over d_in
            # pairs; psum accumulates 32*v; cast to fp8 keeps the 32x scale.
            ps_v = ps_pool.tile([128, 1024], f32, tag="ps")
            for p in range(3):
                for n0, n1 in ((0, 512), (512, DV)):
                    nc.tensor.matmul(
                        ps_v[:, n0:n1],
                        x8o_sb[:, p, :, t * 128 : (t + 1) * 128],
                        wv8_sb[:, 2 * p : 2 * p + 2, n0:n1],
                        start=(p == 0),
                        stop=(p == 2),
                        perf_mode=DR,
                    )
            nc.vector.tensor_copy(v8_sb[:, t, 0:DV], ps_v[:, 0:DV])

        # qc pairs share one SBUF tile and one store DMA; the last two tiles
        # store solo/split so the drain tail is short.
        o_state = {}

        def emit_out_qc(qc):
            # partial numerator + rowsum: out[qc] = sum_t P'8[t,qc].T @ [32v|1]
            # fp8 DoubleRow, t-chunk pairs in the two slots.
            ps_o = ps_pool.tile([128, 1024], f32, tag="ps")
            if qc % 2 == 0:
                o_pair = out_pool.tile([128, 2, DV + 1], bf16, tag="o")
                o_state["tile"] = o_pair
            o_sb = o_state["tile"][:, qc % 2, :]
            # t-pair-major: both col regions back-to-back with the SAME
            # stationary pT8 weights, so the 256-row ldweights of the narrow
            # region is elided/overlapped instead of exposed (~135ns/inst).
            for tp in range(4):
                for n0, n1 in ((0, 512), (512, DV + 2)):
                    nc.tensor.matmul(
                        ps_o[:, n0:n1],
                        pT8_sb[:, 2 * tp : 2 * tp + 2, qc * 128 : (qc + 1) * 128],
                        v8_sb[:, 2 * tp : 2 * tp + 2, n0:n1],
                        start=(tp == 0),
                        stop=(tp == 3),
                        perf_mode=DR,
                    )
            for n0, n1 in ((0, 512), (512, DV + 2)):
                c1 = min(n1, DV + 1)
                nc.vector.tensor_copy(o_sb[:, n0:c1], ps_o[:, n0:c1])
                if qc >= QCH - 2:
                    # last two tiles: store each region immediately, split
                    # by partition across both rings
                    nc.sync.dma_start(
                        out=out[0:64, qc, n0:c1], in_=o_sb[0:64, n0:c1]
                    )
                    nc.scalar.dma_start(
                        out=out[64:128, qc, n0:c1], in_=o_sb[64:128, n0:c1]
                    )
                elif qc % 2 == 1 and n0 == 512:
                    # pair complete: one contiguous 2-tile store
                    eng = nc.sync if (qc // 2) % 2 == 0 else nc.scalar
                    eng.dma_start(
                        out=out[:, qc - 1 : qc + 1, :], in_=o_state["tile"][:]
                    )

        # q own-half then k own, each a single run of region-alternating mms
        # into ONE psum tile (psum switches cost a PE pipeline flush).
        # Chunk order matches DMA arrival order.
        ps_q0 = ps_pool.tile([128, 1024], f32, tag="ps")
        ps_k = ps_pool.tile([128, 1024], f32, tag="ps")
        C_ORDER = [0, 1, 2, 3, 4, 5]
        for dst, w0 in ((ps_q0, 0), (ps_k, DK)):
            for i, c in enumerate(C_ORDER):
                for n0 in (0, 512):
                    nc.tensor.matmul(
                        dst[:, n0 : n0 + 512],
                        wqk_sb[:, c, w0 : w0 + DK],
                        xo(c)[:, n0 : n0 + 512],
                        start=(i == 0),
                        stop=(i == CH - 1),
                    )
        # qT = q̂+bq on scalar, split per 512 so scores t=0 unblocks early
        for n0 in (0, 512):
            nc.scalar.activation(
                qT_sb[:, n0 : n0 + 512],
                ps_q0[:, n0 : n0 + 512],
                mybir.ActivationFunctionType.Identity,
                bias=bq_sb[:],
            )

        # scores for own queries interleaved with v-projection
        for t in range(TCH):
            nc.vector.tensor_copy(
                kT_sb[:, t * 128 : (t + 1) * 128], ps_k[:, t * 128 : (t + 1) * 128]
            )
            emit_scores_t(t, 0)
            emit_v_t(t)

        # q other-half
        ps_q1 = ps_pool.tile([128, 1024], f32, tag="ps")
        for c in range(CH):
            for n0 in (0, 512):
                nc.tensor.matmul(
                    ps_q1[:, n0 : n0 + 512],
                    wqk_sb[:, c, 0:DK],
                    xt_sb[:, c, n0 : n0 + 512],
                    start=(c == 0),
                    stop=(c == CH - 1),
                )
        for n0 in (0, 512):
            nc.scalar.activation(
                qT_sb[:, TOWN + n0 : TOWN + n0 + 512],
                ps_q1[:, n0 : n0 + 512],
                mybir.ActivationFunctionType.Identity,
                bias=bq_sb[:],
            )

        # scores for other-half queries interleaved with out
        for qc in range(8):
            emit_scores_t(qc, 1)
            emit_out_qc(qc)

        for qc in range(8, 16):
            emit_out_qc(qc)

    nc.compile()
    return nc


def _get_nc():
    if "nc" not in _CACHE:
        _CACHE["nc"] = _build()
    return _CACHE["nc"]


def _make_in_maps(x, Wq, bq, Wk, bk, Wv):
    bf16 = ml_dtypes.bfloat16
    fp8 = ml_dtypes.float8_e4m3
    wq = np.asarray(Wq, np.float32).astype(bf16).reshape(CH, 128, DK)
    wk = np.asarray(Wk, np.float32).astype(bf16).reshape(CH, 128, DK)
    base = {
        "wqk": np.ascontiguousarray(
            np.concatenate([wq, wk], axis=2).transpose(1, 0, 2)
        ),
        "wv8": np.ascontiguousarray(
            (np.asarray(Wv, np.float32) * VSCALE)
            .astype(fp8)
            .reshape(CH, 128, DV)
            .transpose(1, 0, 2)
        ),
        "bq": np.ascontiguousarray(np.asarray(bq, np.float32).reshape(DK, 1)),
    }
    in_maps = []
    for c in range(NCORES):
        b, h = c // 2, c % 2
        xb = x[b]  # [T, DIN]
        rot = np.concatenate([xb[h * TOWN :], xb[: h * TOWN]], axis=0)
        xT = rot.T.astype(bf16).reshape(CH, 128, T).transpose(1, 0, 2)  # [128,c,t]
        own = xT[:, :, 0:TOWN]  # [128, c, 1024]
        m = dict(base)
        m["x_own"] = np.ascontiguousarray(own.reshape(128, 3, 2 * TOWN))
        m["x_oth"] = np.ascontiguousarray(xT[:, :, TOWN:T])
        in_maps.append(m)
    return in_maps


def kernel(x, Wq, bq, Wk, bk, Wv, bv):
    from concourse import bass_utils

    x = np.ascontiguousarray(np.asarray(x, dtype=np.float32))
    nc = _get_nc()
    in_maps = _make_in_maps(x, Wq, bq, Wk, bk, Wv)

    res = bass_utils.run_bass_kernel_spmd(nc, in_maps, core_ids=list(range(NCORES)))

    x64 = np.asarray(x, np.float64)
    Wv64 = np.asarray(Wv, np.float64)
    bv64 = np.asarray(bv, np.float64).reshape(1, DV)
    outp = np.empty((B, T, DV), dtype=np.float32)
    for b in range(B):
        # out is partition-major [128, qc, 769] -> [qc*128+p, 769]
        p0 = res.results[2 * b]["out"].transpose(1, 0, 2).reshape(T, DV + 1)
        p1 = res.results[2 * b + 1]["out"].transpose(1, 0, 2).reshape(T, DV + 1)
        p1 = np.concatenate([p1[TOWN:], p1[:TOWN]], axis=0)
        s = p0.astype(np.float64) + p1.astype(np.float64)
        colsum = x64[b].sum(axis=0) @ Wv64  # exact f64 restore of the
        num = s[:, 0:DV] / VSCALE + colsum[None, :]  # dropped "+1" in expm1
        den = s[:, DV : DV + 1] + float(T)
        outp[b] = (num / den + bv64).astype(np.float32)
    return outp


# revision 22
# speedup vs baseline: 1.0464x; 1.0464x over previous
"""Trainium2 Bass kernel for nn_AttentionLayer (4x2048x768, d_k=128, d_v=768).

Sharding (sequence-parallel over keys, data-parallel over batch):
8 cores; core c handles batch b=c//2 with KEY half h=c%2. Each core computes
q for ALL 2048 queries but k/v only for its own 1024 keys, then produces the
partial (unnormalized) attention numerator plus the partial softmax row sum.

fp8 DoubleRow acceleration (2x PE throughput, 256-deep contraction/inst) for
the two dominant matmuls (v-projection and attn@V numerator), with two error
mitigations that keep rel err ~1.1e-2 (< 2e-2 gate):

  1. expm1 trick: the matmul uses P' = exp(s) - 1 quantized to fp8e4 instead
     of exp(s). Softmax weights here are ~1 +- 0.35, so |P'| << |P| and the
     fp8 quantization error shrinks ~3x. The dropped "1" contributes
     colsum_v[n] = sum_t v[t,n] to every query's numerator and T to every
     row sum; both are restored EXACTLY on the host:
         out = (colsum_v + sum_cores P'8@v8/32) / (2048 + sum_cores P'8@1)
     with colsum_v = (sum_t x_t) @ Wv computed in f64 (tiny: 768x768).
  2. The same colsum restore also cancels the common-mode (p-bar-weighted)
     component of the v-side fp8 quantization error.

Numerics per core:
  q/k proj + scores: bf16 (score accuracy dominates overall error).
  exp -> P_hi bf16 (scalar ACT engine, same cost as baseline), then
  gpsimd computes P'8 = fp8(P_hi - 1).
  v-proj: x8 fp8 x wv8 (=32*Wv in fp8) DoubleRow -> psum = 32*v ->
  v8 = fp8(32*v) (vector cast). The 32x pre-scale keeps Wv's tiny uniform
  (+-0.036) values out of fp8e4's subnormal range; host divides by 32.
  numerator: P'8 x v8 DoubleRow, t-chunk pairs packed in the two slots.
  bk is dropped (softmax-invariant); bq/bv handled as in the baseline.

DMA: inputs host-repacked partition-major into few fat-row DMAs on the
scalar + sync rings (~0.17MB/us each), ordered by first PE use with WAW
gates so early q/k loads aren't starved. Output partial [128, 16, 769]
bf16, stored in qc pairs alternating rings.
"""

import sys

sys.path.insert(0, "/opt/trn_rl_repo")

import numpy as np
import ml_dtypes

B, T, DIN, DK, DV = 4, 2048, 768, 128, 768
NCORES = 8
TOWN = 1024  # own keys per core
CH = DIN // 128  # 6 contraction chunks over d_in
TCH = TOWN // 128  # 8 own-key chunks
QCH = T // 128  # 16 query chunks (all queries)
SCALE = 1.0 / float(np.sqrt(DK))
VSCALE = 32.0  # fp8 pre-scale on Wv (power of 2; host divides out)

_CACHE = {}


def _build():
    from contextlib import ExitStack

    from concourse import bacc, mybir, tile

    f32 = mybir.dt.float32
    bf16 = mybir.dt.bfloat16
    fp8 = mybir.dt.float8e4
    DR = mybir.MatmulPerfMode.DoubleRow

    nc = bacc.Bacc("TRN2", target_bir_lowering=False, debug=False)

    x_own = nc.dram_tensor("x_own", [128, 3, 2 * TOWN], bf16, kind="ExternalInput").ap()
    x_oth = nc.dram_tensor("x_oth", [128, CH, TOWN], bf16, kind="ExternalInput").ap()
    wqk = nc.dram_tensor("wqk", [128, CH, 2 * DK], bf16, kind="ExternalInput").ap()
    wv8 = nc.dram_tensor("wv8", [128, CH, DV], fp8, kind="ExternalInput").ap()
    bq = nc.dram_tensor("bq", [DK, 1], f32, kind="ExternalInput").ap()
    out = nc.dram_tensor("out", [128, QCH, DV + 1], bf16, kind="ExternalOutput").ap()

    with tile.TileContext(nc) as tc, ExitStack() as ctx:
        consts = ctx.enter_context(tc.tile_pool(name="consts", bufs=1))
        persist = ctx.enter_context(tc.tile_pool(name="persist", bufs=1))
        wpool = ctx.enter_context(tc.tile_pool(name="wpool", bufs=1))
        xpool = ctx.enter_context(tc.tile_pool(name="xpool", bufs=1))
        ph_pool = ctx.enter_context(tc.tile_pool(name="ph", bufs=3))
        out_pool = ctx.enter_context(tc.tile_pool(name="out_pool", bufs=4))
        ps_pool = ctx.enter_context(tc.tile_pool(name="ps", bufs=3, space="PSUM"))
        sc_pool = ctx.enter_context(tc.tile_pool(name="sc", bufs=2, space="PSUM"))

        bq_sb = consts.tile([DK, 1], f32)
        nc.gpsimd.dma_start(out=bq_sb[:], in_=bq)

        qT_sb = persist.tile([128, T], bf16)  # [dk, q] all queries, q̂+bq
        kT_sb = persist.tile([128, TOWN], bf16)  # [dk, t-own]
        v8_sb = persist.tile([128, TCH, 1024], fp8)  # [t-part, chunk, 32v|1|0pad]
        pT8_sb = persist.tile([128, TCH, T], fp8)  # [t-part, chunk, q] = exp-1

        nc.vector.memset(v8_sb[:, :, DV : DV + 1], 1.0)
        nc.vector.memset(v8_sb[:, :, DV + 1 : 1024], 0.0)

        # x-own pairs: pair p holds chunks 2p|2p+1 side by side (4KB rows)
        xo_sb = xpool.tile([128, 3, 2 * TOWN], bf16)
        xt_sb = xpool.tile([128, CH, TOWN], bf16)
        x8o_sb = xpool.tile([128, 3, 2, TOWN], fp8)
        wqk_sb = wpool.tile([128, CH, 2 * DK], bf16)
        wv8_sb = wpool.tile([128, CH, 1024], fp8)  # cols DV..1023 zero-padded
        nc.vector.memset(wv8_sb[:, :, DV:1024], 0.0)

        def xo(c):  # own-x chunk c -> [128, TOWN] slice of the pair tile
            return xo_sb[:, c // 2, (c % 2) * TOWN : (c % 2 + 1) * TOWN]

        # Input DMAs: x pairs land in q/k consumption order (pair0 sync,
        # pair1 scalar, pair2 sync). Each ring's DMAs are serialized by WAW
        # gates (the rings round-robin over all queued DMAs, so an ungated
        # late load steals bandwidth from the critical early ones). The fp8
        # copy of own-x (for the v-proj DoubleRow) is cast on-chip by the
        # vector engine, which is idle during q/k.
        nc.scalar.dma_start(out=wqk_sb[:], in_=wqk)
        nc.sync.dma_start(out=xo_sb[:, 0, :], in_=x_own[:, 0, :])
        nc.scalar.dma_start(out=xo_sb[:, 1, :], in_=x_own[:, 1, :])
        nc.sync.dma_start(out=xo_sb[:, 2, :], in_=x_own[:, 2, :])
        nc.vector.tensor_copy(wv8_sb[:, 0, 0:1], xo_sb[:, 0, 0:1])
        nc.scalar.dma_start(out=wv8_sb[:, :, 0:DV], in_=wv8)
        nc.vector.tensor_copy(xt_sb[:, 0, 0:1], xo_sb[:, 2, 0:1])
        nc.vector.tensor_copy(xt_sb[:, 3, 0:1], wv8_sb[:, 0, 1:2])
        nc.sync.dma_start(out=xt_sb[:, 0:3, :], in_=x_oth[:, 0:3, :])
        nc.scalar.dma_start(out=xt_sb[:, 3:6, :], in_=x_oth[:, 3:6, :])
        # on-chip bf16 -> fp8 casts of own-x, one per d_in chunk
        for p in range(3):
            for s in range(2):
                nc.vector.tensor_copy(
                    x8o_sb[:, p, s, :], xo_sb[:, p, s * TOWN : (s + 1) * TOWN]
                )

        def emit_scores_t(t, qh):
            # scores^T for one own-key chunk -> P'8 = fp8(exp(scale*s) - 1).
            # Two 512-col exps pipeline from two psum banks into one 1024-col
            # ph tile; a single 1024-col vector op does the -1 + fp8 cast.
            ph = ph_pool.tile([128, 1024], bf16, tag="ph")
            for n0 in (0, 512):
                ps_s = sc_pool.tile([128, 512], f32, tag="sc")
                nc.tensor.matmul(
                    ps_s[:],
                    kT_sb[:, t * 128 : (t + 1) * 128],
                    qT_sb[:, qh * 1024 + n0 : qh * 1024 + n0 + 512],
                    start=True,
                    stop=True,
                )
                nc.scalar.activation(
                    ph[:, n0 : n0 + 512],
                    ps_s[:],
                    mybir.ActivationFunctionType.Exp,
                    scale=SCALE,
                )
            nc.vector.tensor_scalar(
                out=pT8_sb[:, t, qh * 1024 : qh * 1024 + 1024],
                in0=ph[:],
                scalar1=1.0,
                scalar2=None,
                op0=mybir.AluOpType.subtract,
            )

        def emit_v_t(t):
            # v-projection for one own-key chunk: fp8 DoubleRow over d_in
            # pairs; psum accumulates 32*v; cast to fp8 keeps the 32x scale.
            ps_v = ps_pool.tile([128, 1024], f32, tag="ps")
            for p in range(3):
                for n0, n1 in ((0, 512), (512, 1024)):
                    nc.tensor.matmul(
                        ps_v[:, n0:n1],
                        x8o_sb[:, p, :, t * 128 : (t + 1) * 128],
                        wv8_sb[:, 2 * p : 2 * p + 2, n0:n1],
                        start=(p == 0),
                        stop=(p == 2),
                        perf_mode=DR,
                    )
            nc.vector.tensor_copy(v8_sb[:, t, 0:DV], ps_v[:, 0:DV])

        # qc pairs share one SBUF tile and one store DMA; the last two tiles
        # store solo/split so the drain tail is short.
        o_state = {}

        def emit_out_qc(qc):
            # partial numerator + rowsum: out[qc] = sum_t P'8[t,qc].T @ [32v|1]
            # fp8 DoubleRow, t-chunk pairs in the two slots.
            ps_o = ps_pool.tile([128, 1024], f32, tag="ps")
            if qc % 2 == 0:
                o_pair = out_pool.tile([128, 2, DV + 1], bf16, tag="o")
                o_state["tile"] = o_pair
            o_sb = o_state["tile"][:, qc % 2, :]
            # both regions 512 wide (v8 zero-padded): narrow DR matmuls are
            # ldweights-bound (~379ns vs 273ns for 512-col), so padding the
            # second region with zero columns is net faster.
            for n0, n1 in ((0, 512), (512, 1024)):
                for tp in range(4):
                    nc.tensor.matmul(
                        ps_o[:, n0:n1],
                        pT8_sb[:, 2 * tp : 2 * tp + 2, qc * 128 : (qc + 1) * 128],
                        v8_sb[:, 2 * tp : 2 * tp + 2, n0:n1],
                        start=(tp == 0),
                        stop=(tp == 3),
                        perf_mode=DR,
                    )
            for n0, n1 in ((0, 512), (512, DV + 2)):
                c1 = min(n1, DV + 1)
                nc.vector.tensor_copy(o_sb[:, n0:c1], ps_o[:, n0:c1])
                if qc >= QCH - 2:
                    # last two tiles: store each region immediately, split
                    # by partition across both rings
                    nc.sync.dma_start(
                        out=out[0:64, qc, n0:c1], in_=o_sb[0:64, n0:c1]
                    )
                    nc.scalar.dma_start(
                        out=out[64:128, qc, n0:c1], in_=o_sb[64:128, n0:c1]
                    )
                elif qc % 2 == 1 and n0 == 512:
                    # pair complete: one contiguous 2-tile store
                    eng = nc.sync if (qc // 2) % 2 == 0 else nc.scalar
                    eng.dma_start(
                        out=out[:, qc - 1 : qc + 1, :], in_=o_state["tile"][:]
                    )

        # q own-half then k own, each a single run of region-alternating mms
        # into ONE psum tile (psum switches cost a PE pipeline flush).
        # Chunk order matches DMA arrival order.
        ps_q0 = ps_pool.tile([128, 1024], f32, tag="ps")
        ps_k = ps_pool.tile([128, 1024], f32, tag="ps")
        C_ORDER = [0, 1, 2, 3, 4, 5]
        for dst, w0 in ((ps_q0, 0), (ps_k, DK)):
            for i, c in enumerate(C_ORDER):
                for n0 in (0, 512):
                    nc.tensor.matmul(
                        dst[:, n0 : n0 + 512],
                        wqk_sb[:, c, w0 : w0 + DK],
                        xo(c)[:, n0 : n0 + 512],
                        start=(i == 0),
                        stop=(i == CH - 1),
                    )
        # qT = q̂+bq on scalar, split per 512 so scores t=0 unblocks early
        for n0 in (0, 512):
            nc.scalar.activation(
                qT_sb[:, n0 : n0 + 512],
                ps_q0[:, n0 : n0 + 512],
                mybir.ActivationFunctionType.Identity,
                bias=bq_sb[:],
            )

        # scores for own queries interleaved with v-projection
        for t in range(TCH):
            nc.vector.tensor_copy(
                kT_sb[:, t * 128 : (t + 1) * 128], ps_k[:, t * 128 : (t + 1) * 128]
            )
            emit_scores_t(t, 0)
            emit_v_t(t)

        # q other-half
        ps_q1 = ps_pool.tile([128, 1024], f32, tag="ps")
        for c in range(CH):
            for n0 in (0, 512):
                nc.tensor.matmul(
                    ps_q1[:, n0 : n0 + 512],
                    wqk_sb[:, c, 0:DK],
                    xt_sb[:, c, n0 : n0 + 512],
                    start=(c == 0),
                    stop=(c == CH - 1),
                )
        for n0 in (0, 512):
            nc.scalar.activation(
                qT_sb[:, TOWN + n0 : TOWN + n0 + 512],
                ps_q1[:, n0 : n0 + 512],
                mybir.ActivationFunctionType.Identity,
                bias=bq_sb[:],
            )

        # scores for other-half queries interleaved with out
        for qc in range(8):
            emit_scores_t(qc, 1)
            emit_out_qc(qc)

        for qc in range(8, 16):
            emit_out_qc(qc)

    nc.compile()
    return nc


def _get_nc():
    if "nc" not in _CACHE:
        _CACHE["nc"] = _build()
    return _CACHE["nc"]


def _make_in_maps(x, Wq, bq, Wk, bk, Wv):
    bf16 = ml_dtypes.bfloat16
    fp8 = ml_dtypes.float8_e4m3
    wq = np.asarray(Wq, np.float32).astype(bf16).reshape(CH, 128, DK)
    wk = np.asarray(Wk, np.float32).astype(bf16).reshape(CH, 128, DK)
    base = {
        "wqk": np.ascontiguousarray(
            np.concatenate([wq, wk], axis=2).transpose(1, 0, 2)
        ),
        "wv8": np.ascontiguousarray(
            (np.asarray(Wv, np.float32) * VSCALE)
            .astype(fp8)
            .reshape(CH, 128, DV)
            .transpose(1, 0, 2)
        ),
        "bq": np.ascontiguousarray(np.asarray(bq, np.float32).reshape(DK, 1)),
    }
    in_maps = []
    for c in range(NCORES):
        b, h = c // 2, c % 2
        xb = x[b]  # [T, DIN]
        rot = np.concatenate([xb[h * TOWN :], xb[: h * TOWN]], axis=0)
        xT = rot.T.astype(bf16).reshape(CH, 128, T).transpose(1, 0, 2)  # [128,c,t]
        own = xT[:, :, 0:TOWN]  # [128, c, 1024]
        m = dict(base)
        m["x_own"] = np.ascontiguousarray(own.reshape(128, 3, 2 * TOWN))
        m["x_oth"] = np.ascontiguousarray(xT[:, :, TOWN:T])
        in_maps.append(m)
    return in_maps


def kernel(x, Wq, bq, Wk, bk, Wv, bv):
    from concourse import bass_utils

    x = np.ascontiguousarray(np.asarray(x, dtype=np.float32))
    nc = _get_nc()
    in_maps = _make_in_maps(x, Wq, bq, Wk, bk, Wv)

    res = bass_utils.run_bass_kernel_spmd(nc, in_maps, core_ids=list(range(NCORES)))

    x64 = np.asarray(x, np.float64)
    Wv64 = np.asarray(Wv, np.float64)
    bv64 = np.asarray(bv, np.float64).reshape(1, DV)
    outp = np.empty((B, T, DV), dtype=np.float32)
    for b in range(B):
        # out is partition-major [128, qc, 769] -> [qc*128+p, 769]
        p0 = res.results[2 * b]["out"].transpose(1, 0, 2).reshape(T, DV + 1)
        p1 = res.results[2 * b + 1]["out"].transpose(1, 0, 2).reshape(T, DV + 1)
        p1 = np.concatenate([p1[TOWN:], p1[:TOWN]], axis=0)
        s = p0.astype(np.float64) + p1.astype(np.float64)
        colsum = x64[b].sum(axis=0) @ Wv64  # exact f64 restore of the
        num = s[:, 0:DV] / VSCALE + colsum[None, :]  # dropped "+1" in expm1
        den = s[:, DV : DV + 1] + float(T)
        outp[b] = (num / den + bv64).astype(np.float32)
    return outp


# revision 23
# speedup vs baseline: 1.1365x; 1.0861x over previous
"""Trainium2 Bass kernel for nn_AttentionLayer (4x2048x768, d_k=128, d_v=768).

Sharding (sequence-parallel over keys, data-parallel over batch):
8 cores; core c handles batch b=c//2 with KEY half h=c%2. Each core computes
q for ALL 2048 queries but k/v only for its own 1024 keys, then produces the
partial (unnormalized) attention numerator plus the partial softmax row sum.

fp8 DoubleRow acceleration (2x PE throughput, 256-deep contraction/inst) for
the two dominant matmuls (v-projection and attn@V numerator), with two error
mitigations that keep rel err ~1.1e-2 (< 2e-2 gate):

  1. expm1 trick: the matmul uses P' = exp(s) - 1 quantized to fp8e4 instead
     of exp(s). Softmax weights here are ~1 +- 0.35, so |P'| << |P| and the
     fp8 quantization error shrinks ~3x. The dropped "1" contributes
     colsum_v[n] = sum_t v[t,n] to every query's numerator and T to every
     row sum; both are restored EXACTLY on the host:
         out = (colsum_v + sum_cores P'8@v8/32) / (2048 + sum_cores P'8@1)
     with colsum_v = (sum_t x_t) @ Wv computed in f64 (tiny: 768x768).
  2. The same colsum restore also cancels the common-mode (p-bar-weighted)
     component of the v-side fp8 quantization error.

Numerics per core:
  q/k proj + scores: bf16 (score accuracy dominates overall error).
  exp -> P_hi bf16 (scalar ACT engine, same cost as baseline), then
  gpsimd computes P'8 = fp8(P_hi - 1).
  v-proj: x8 fp8 x wv8 (=32*Wv in fp8) DoubleRow -> psum = 32*v ->
  v8 = fp8(32*v) (vector cast). The 32x pre-scale keeps Wv's tiny uniform
  (+-0.036) values out of fp8e4's subnormal range; host divides by 32.
  numerator: P'8 x v8 DoubleRow, t-chunk pairs packed in the two slots.
  bk is dropped (softmax-invariant); bq/bv handled as in the baseline.

DMA: inputs host-repacked partition-major into few fat-row DMAs on the
scalar + sync rings (~0.17MB/us each), ordered by first PE use with WAW
gates so early q/k loads aren't starved. Output partial [128, 16, 769]
bf16, stored in qc pairs alternating rings.
"""

import sys

sys.path.insert(0, "/opt/trn_rl_repo")

import numpy as np
import ml_dtypes

B, T, DIN, DK, DV = 4, 2048, 768, 128, 768
NCORES = 8
TOWN = 1024  # own keys per core
CH = DIN // 128  # 6 contraction chunks over d_in
TCH = TOWN // 128  # 8 own-key chunks
QCH = T // 128  # 16 query chunks (all queries)
SCALE = 1.0 / float(np.sqrt(DK))
VSCALE = 32.0  # fp8 pre-scale on Wv (power of 2; host divides out)

_CACHE = {}


def _build():
    from contextlib import ExitStack

    from concourse import bacc, mybir, tile

    f32 = mybir.dt.float32
    bf16 = mybir.dt.bfloat16
    fp8 = mybir.dt.float8e4
    DR = mybir.MatmulPerfMode.DoubleRow

    nc = bacc.Bacc("TRN2", target_bir_lowering=False, debug=False)

    x_own = nc.dram_tensor("x_own", [128, 3, 2 * TOWN], bf16, kind="ExternalInput").ap()
    x_oth = nc.dram_tensor("x_oth", [128, CH, TOWN], bf16, kind="ExternalInput").ap()
    wqk = nc.dram_tensor("wqk", [128, CH, 2 * DK], bf16, kind="ExternalInput").ap()
    wv8 = nc.dram_tensor("wv8", [128, CH, DV], fp8, kind="ExternalInput").ap()
    bq = nc.dram_tensor("bq", [DK, 1], f32, kind="ExternalInput").ap()
    out = nc.dram_tensor("out", [128, QCH, DV + 1], bf16, kind="ExternalOutput").ap()

    with tile.TileContext(nc) as tc, ExitStack() as ctx:
        consts = ctx.enter_context(tc.tile_pool(name="consts", bufs=1))
        persist = ctx.enter_context(tc.tile_pool(name="persist", bufs=1))
        wpool = ctx.enter_context(tc.tile_pool(name="wpool", bufs=1))
        xpool = ctx.enter_context(tc.tile_pool(name="xpool", bufs=1))
        ph_pool = ctx.enter_context(tc.tile_pool(name="ph", bufs=3))
        out_pool = ctx.enter_context(tc.tile_pool(name="out_pool", bufs=4))
        ps_pool = ctx.enter_context(tc.tile_pool(name="ps", bufs=3, space="PSUM"))
        sc_pool = ctx.enter_context(tc.tile_pool(name="sc", bufs=2, space="PSUM"))

        bq_sb = consts.tile([DK, 1], f32)
        nc.gpsimd.dma_start(out=bq_sb[:], in_=bq)

        qT_sb = persist.tile([128, T], bf16)  # [dk, q] all queries, q̂+bq
        kT_sb = persist.tile([128, TOWN], bf16)  # [dk, t-own]
        v8_sb = persist.tile([128, TCH, DV + 2], fp8)  # [t-part, chunk, 32v|1|0]
        pT8_sb = persist.tile([128, TCH, T], fp8)  # [t-part, chunk, q] = exp-1

        nc.vector.memset(v8_sb[:, :, DV : DV + 1], 1.0)
        nc.vector.memset(v8_sb[:, :, DV + 1 : DV + 2], 0.0)

        # x-own pairs: pair p holds chunks 2p|2p+1 side by side (4KB rows)
        xo_sb = xpool.tile([128, 3, 2 * TOWN], bf16)
        xt_sb = xpool.tile([128, CH, TOWN], bf16)
        x8o_sb = xpool.tile([128, 3, 2, TOWN], fp8)
        wqk_sb = wpool.tile([128, CH, 2 * DK], bf16)
        wv8_sb = wpool.tile([128, CH, DV], fp8)

        def xo(c):  # own-x chunk c -> [128, TOWN] slice of the pair tile
            return xo_sb[:, c // 2, (c % 2) * TOWN : (c % 2 + 1) * TOWN]

        # Input DMAs: x pairs land in q/k consumption order (pair0 sync,
        # pair1 scalar, pair2 sync). Each ring's DMAs are serialized by WAW
        # gates (the rings round-robin over all queued DMAs, so an ungated
        # late load steals bandwidth from the critical early ones). The fp8
        # copy of own-x (for the v-proj DoubleRow) is cast on-chip by the
        # vector engine, which is idle during q/k.
        nc.scalar.dma_start(out=wqk_sb[:], in_=wqk)

        def xo_dram(c):
            return x_own[:, c // 2, (c % 2) * TOWN : (c % 2 + 1) * TOWN]

        # per-chunk x loads (0.26MB each): chunk0 lands ~2us earlier than a
        # whole-pair DMA would, so q/k starts sooner. sync: c0,c1,c4,c5;
        # scalar: c2,c3 (after wqk) - matches consumption order 0..5.
        for c in (0, 1):
            nc.sync.dma_start(out=xo(c), in_=xo_dram(c))
        for c in (2, 3):
            nc.scalar.dma_start(out=xo(c), in_=xo_dram(c))
        for c in (4, 5):
            nc.sync.dma_start(out=xo(c), in_=xo_dram(c))
        nc.vector.tensor_copy(wv8_sb[:, 0, 0:1], xo_sb[:, 0, 0:1])
        nc.scalar.dma_start(out=wv8_sb[:], in_=wv8)
        nc.vector.tensor_copy(xt_sb[:, 0, 0:1], xo_sb[:, 2, 0:1])
        nc.vector.tensor_copy(xt_sb[:, 3, 0:1], wv8_sb[:, 0, 1:2])
        nc.sync.dma_start(out=xt_sb[:, 0:3, :], in_=x_oth[:, 0:3, :])
        nc.scalar.dma_start(out=xt_sb[:, 3:6, :], in_=x_oth[:, 3:6, :])
        # on-chip bf16 -> fp8 casts of own-x, one per d_in chunk
        for p in range(3):
            for s in range(2):
                nc.vector.tensor_copy(
                    x8o_sb[:, p, s, :], xo_sb[:, p, s * TOWN : (s + 1) * TOWN]
                )

        def emit_scores_t(t, qh):
            # scores^T for one own-key chunk -> P'8 = fp8(exp(scale*s) - 1).
            # Two 512-col exps pipeline from two psum banks into one 1024-col
            # ph tile; a single 1024-col vector op does the -1 + fp8 cast.
            ph = ph_pool.tile([128, 1024], bf16, tag="ph")
            for n0 in (0, 512):
                ps_s = sc_pool.tile([128, 512], f32, tag="sc")
                nc.tensor.matmul(
                    ps_s[:],
                    kT_sb[:, t * 128 : (t + 1) * 128],
                    qT_sb[:, qh * 1024 + n0 : qh * 1024 + n0 + 512],
                    start=True,
                    stop=True,
                )
                nc.scalar.activation(
                    ph[:, n0 : n0 + 512],
                    ps_s[:],
                    mybir.ActivationFunctionType.Exp,
                    scale=SCALE,
                )
            nc.vector.tensor_scalar(
                out=pT8_sb[:, t, qh * 1024 : qh * 1024 + 1024],
                in0=ph[:],
                scalar1=1.0,
                scalar2=None,
                op0=mybir.AluOpType.subtract,
            )

        def emit_v_t(t):
            # v-projection for one own-key chunk: fp8 DoubleRow over d_in
            # pairs; psum accumulates 32*v; cast to fp8 keeps the 32x scale.
            ps_v = ps_pool.tile([128, 1024], f32, tag="ps")
            for p in range(3):
                for n0, n1 in ((0, 512), (512, DV)):
                    nc.tensor.matmul(
                        ps_v[:, n0:n1],
                        x8o_sb[:, p, :, t * 128 : (t + 1) * 128],
                        wv8_sb[:, 2 * p : 2 * p + 2, n0:n1],
                        start=(p == 0),
                        stop=(p == 2),
                        perf_mode=DR,
                    )
            nc.vector.tensor_copy(v8_sb[:, t, 0:DV], ps_v[:, 0:DV])

        # qc pairs share one SBUF tile and one store DMA; the last two tiles
        # store solo/split so the drain tail is short.
        o_state = {}

        def emit_out_qc(qc):
            # partial numerator + rowsum: out[qc] = sum_t P'8[t,qc].T @ [32v|1]
            # fp8 DoubleRow, t-chunk pairs in the two slots.
            ps_o = ps_pool.tile([128, 1024], f32, tag="ps")
            if qc % 2 == 0:
                o_pair = out_pool.tile([128, 2, DV + 1], bf16, tag="o")
                o_state["tile"] = o_pair
            o_sb = o_state["tile"][:, qc % 2, :]
            for n0, n1 in ((0, 512), (512, DV + 2)):
                for tp in range(4):
                    nc.tensor.matmul(
                        ps_o[:, n0:n1],
                        pT8_sb[:, 2 * tp : 2 * tp + 2, qc * 128 : (qc + 1) * 128],
                        v8_sb[:, 2 * tp : 2 * tp + 2, n0:n1],
                        start=(tp == 0),
                        stop=(tp == 3),
                        perf_mode=DR,
                    )
            for n0, n1 in ((0, 512), (512, DV + 2)):
                c1 = min(n1, DV + 1)
                nc.vector.tensor_copy(o_sb[:, n0:c1], ps_o[:, n0:c1])
                if qc >= QCH - 2:
                    # last two tiles: store each region immediately, split
                    # by partition across both rings
                    nc.sync.dma_start(
                        out=out[0:64, qc, n0:c1], in_=o_sb[0:64, n0:c1]
                    )
                    nc.scalar.dma_start(
                        out=out[64:128, qc, n0:c1], in_=o_sb[64:128, n0:c1]
                    )
                elif qc % 2 == 1 and n0 == 512:
                    # pair complete: one contiguous 2-tile store
                    eng = nc.sync if (qc // 2) % 2 == 0 else nc.scalar
                    eng.dma_start(
                        out=out[:, qc - 1 : qc + 1, :], in_=o_state["tile"][:]
                    )

        # q own-half then k own, each a single run of region-alternating mms
        # into ONE psum tile (psum switches cost a PE pipeline flush).
        # Chunk order matches DMA arrival order.
        ps_q0 = ps_pool.tile([128, 1024], f32, tag="ps")
        ps_k = ps_pool.tile([128, 1024], f32, tag="ps")
        C_ORDER = [0, 1, 2, 3, 4, 5]
        for dst, w0 in ((ps_q0, 0), (ps_k, DK)):
            for i, c in enumerate(C_ORDER):
                for n0 in (0, 512):
                    nc.tensor.matmul(
                        dst[:, n0 : n0 + 512],
                        wqk_sb[:, c, w0 : w0 + DK],
                        xo(c)[:, n0 : n0 + 512],
                        start=(i == 0),
                        stop=(i == CH - 1),
                    )
        # qT = q̂+bq on scalar, split per 512 so scores t=0 unblocks early
        for n0 in (0, 512):
            nc.scalar.activation(
                qT_sb[:, n0 : n0 + 512],
                ps_q0[:, n0 : n0 + 512],
                mybir.ActivationFunctionType.Identity,
                bias=bq_sb[:],
            )

        # scores for own queries interleaved with v-projection
        for t in range(TCH):
            nc.vector.tensor_copy(
                kT_sb[:, t * 128 : (t + 1) * 128], ps_k[:, t * 128 : (t + 1) * 128]
            )
            emit_scores_t(t, 0)
            emit_v_t(t)

        # q other-half
        ps_q1 = ps_pool.tile([128, 1024], f32, tag="ps")
        for c in range(CH):
            for n0 in (0, 512):
                nc.tensor.matmul(
                    ps_q1[:, n0 : n0 + 512],
                    wqk_sb[:, c, 0:DK],
                    xt_sb[:, c, n0 : n0 + 512],
                    start=(c == 0),
                    stop=(c == CH - 1),
                )
        for n0 in (0, 512):
            nc.scalar.activation(
                qT_sb[:, TOWN + n0 : TOWN + n0 + 512],
                ps_q1[:, n0 : n0 + 512],
                mybir.ActivationFunctionType.Identity,
                bias=bq_sb[:],
            )

        # scores for other-half queries interleaved with out
        for qc in range(8):
            emit_scores_t(qc, 1)
            emit_out_qc(qc)

        for qc in range(8, 16):
            emit_out_qc(qc)

    nc.compile()
    return nc


def _get_nc():
    if "nc" not in _CACHE:
        _CACHE["nc"] = _build()
    return _CACHE["nc"]


def _make_in_maps(x, Wq, bq, Wk, bk, Wv):
    bf16 = ml_dtypes.bfloat16
    fp8 = ml_dtypes.float8_e4m3
    wq = np.asarray(Wq, np.float32).astype(bf16).reshape(CH, 128, DK)
    wk = np.asarray(Wk, np.float32).astype(bf16).reshape(CH, 128, DK)
    base = {
        "wqk": np.ascontiguousarray(
            np.concatenate([wq, wk], axis=2).transpose(1, 0, 2)
        ),
        "wv8": np.ascontiguousarray(
            (np.asarray(Wv, np.float32) * VSCALE)
            .astype(fp8)
            .reshape(CH, 128, DV)
            .transpose(1, 0, 2)
        ),
        "bq": np.ascontiguousarray(np.asarray(bq, np.float32).reshape(DK, 1)),
    }
    in_maps = []
    for c in range(NCORES):
        b, h = c // 2, c % 2
        xb = x[b]  # [T, DIN]
        rot = np.concatenate([xb[h * TOWN :], xb[: h * TOWN]], axis=0)
        xT = rot.T.astype(bf16).reshape(CH, 128, T).transpose(1, 0, 2)  # [128,c,t]
        own = xT[:, :, 0:TOWN]  # [128, c, 1024]
        m = dict(base)
        m["x_own"] = np.ascontiguousarray(own.reshape(128, 3, 2 * TOWN))
        m["x_oth"] = np.ascontiguousarray(xT[:, :, TOWN:T])
        in_maps.append(m)
    return in_maps


def kernel(x, Wq, bq, Wk, bk, Wv, bv):
    from concourse import bass_utils

    x = np.ascontiguousarray(np.asarray(x, dtype=np.float32))
    nc = _get_nc()
    in_maps = _make_in_maps(x, Wq, bq, Wk, bk, Wv)

    res = bass_utils.run_bass_kernel_spmd(nc, in_maps, core_ids=list(range(NCORES)))

    x64 = np.asarray(x, np.float64)
    Wv64 = np.asarray(Wv, np.float64)
    bv64 = np.asarray(bv, np.float64).reshape(1, DV)
    outp = np.empty((B, T, DV), dtype=np.float32)
    for b in range(B):
        # out is partition-major [128, qc, 769] -> [qc*128+p, 769]
        p0 = res.results[2 * b]["out"].transpose(1, 0, 2).reshape(T, DV + 1)
        p1 = res.results[2 * b + 1]["out"].transpose(1, 0, 2).reshape(T, DV + 1)
        p1 = np.concatenate([p1[TOWN:], p1[:TOWN]], axis=0)
        s = p0.astype(np.float64) + p1.astype(np.float64)
        colsum = x64[b].sum(axis=0) @ Wv64  # exact f64 restore of the
        num = s[:, 0:DV] / VSCALE + colsum[None, :]  # dropped "+1" in expm1
        den = s[:, DV : DV + 1] + float(T)
        outp[b] = (num / den + bv64).astype(np.float32)
    return outp


# revision 25
# speedup vs baseline: 1.1800x; 1.0383x over previous
"""Trainium2 Bass kernel for nn_AttentionLayer (4x2048x768, d_k=128, d_v=768).

Sharding (sequence-parallel over keys, data-parallel over batch):
8 cores; core c handles batch b=c//2 with KEY half h=c%2. Each core computes
q for ALL 2048 queries but k/v only for its own 1024 keys, then produces the
partial (unnormalized) attention numerator plus the partial softmax row sum.

fp8 DoubleRow acceleration (2x PE throughput, 256-deep contraction/inst) for
the two dominant matmuls (v-projection and attn@V numerator), with two error
mitigations that keep rel err ~1.1e-2 (< 2e-2 gate):

  1. expm1 trick: the matmul uses P' = exp(s) - 1 quantized to fp8e4 instead
     of exp(s). Softmax weights here are ~1 +- 0.35, so |P'| << |P| and the
     fp8 quantization error shrinks ~3x. The dropped "1" contributes
     colsum_v[n] = sum_t v[t,n] to every query's numerator and T to every
     row sum; both are restored EXACTLY on the host:
         out = (colsum_v + sum_cores P'8@v8/32) / (2048 + sum_cores P'8@1)
     with colsum_v = (sum_t x_t) @ Wv computed in f64 (tiny: 768x768).
  2. The same colsum restore also cancels the common-mode (p-bar-weighted)
     component of the v-side fp8 quantization error.

Numerics per core:
  q/k proj + scores: bf16 (score accuracy dominates overall error).
  exp -> P_hi bf16 (scalar ACT engine, same cost as baseline), then
  gpsimd computes P'8 = fp8(P_hi - 1).
  v-proj: x8 fp8 x wv8 (=32*Wv in fp8) DoubleRow -> psum = 32*v ->
  v8 = fp8(32*v) (vector cast). The 32x pre-scale keeps Wv's tiny uniform
  (+-0.036) values out of fp8e4's subnormal range; host divides by 32.
  numerator: P'8 x v8 DoubleRow, t-chunk pairs packed in the two slots.
  bk is dropped (softmax-invariant); bq/bv handled as in the baseline.

DMA: inputs host-repacked partition-major into few fat-row DMAs on the
scalar + sync rings (~0.17MB/us each), ordered by first PE use with WAW
gates so early q/k loads aren't starved. Output partial [128, 16, 769]
bf16, stored in qc pairs alternating rings.
"""

import sys

sys.path.insert(0, "/opt/trn_rl_repo")

import numpy as np
import ml_dtypes

B, T, DIN, DK, DV = 4, 2048, 768, 128, 768
NCORES = 8
TOWN = 1024  # own keys per core
CH = DIN // 128  # 6 contraction chunks over d_in
TCH = TOWN // 128  # 8 own-key chunks
QCH = T // 128  # 16 query chunks (all queries)
SCALE = 1.0 / float(np.sqrt(DK))
VSCALE = 32.0  # fp8 pre-scale on Wv (power of 2; host divides out)

_CACHE = {}


def _build():
    from contextlib import ExitStack

    from concourse import bacc, mybir, tile

    f32 = mybir.dt.float32
    bf16 = mybir.dt.bfloat16
    fp8 = mybir.dt.float8e4
    DR = mybir.MatmulPerfMode.DoubleRow

    nc = bacc.Bacc("TRN2", target_bir_lowering=False, debug=False)

    x_own = nc.dram_tensor("x_own", [128, 3, 2 * TOWN], bf16, kind="ExternalInput").ap()
    x_oth = nc.dram_tensor("x_oth", [128, CH, TOWN], bf16, kind="ExternalInput").ap()
    wqk = nc.dram_tensor("wqk", [128, CH, 2 * DK], bf16, kind="ExternalInput").ap()
    wv8 = nc.dram_tensor("wv8", [128, CH, DV], fp8, kind="ExternalInput").ap()
    bq = nc.dram_tensor("bq", [DK, 1], f32, kind="ExternalInput").ap()
    out = nc.dram_tensor("out", [128, QCH, DV + 1], fp8, kind="ExternalOutput").ap()

    with tile.TileContext(nc) as tc, ExitStack() as ctx:
        consts = ctx.enter_context(tc.tile_pool(name="consts", bufs=1))
        persist = ctx.enter_context(tc.tile_pool(name="persist", bufs=1))
        wpool = ctx.enter_context(tc.tile_pool(name="wpool", bufs=1))
        xpool = ctx.enter_context(tc.tile_pool(name="xpool", bufs=1))
        ph_pool = ctx.enter_context(tc.tile_pool(name="ph", bufs=3))
        out_pool = ctx.enter_context(tc.tile_pool(name="out_pool", bufs=4))
        ps_pool = ctx.enter_context(tc.tile_pool(name="ps", bufs=3, space="PSUM"))
        sc_pool = ctx.enter_context(tc.tile_pool(name="sc", bufs=2, space="PSUM"))

        bq_sb = consts.tile([DK, 1], f32)
        nc.gpsimd.dma_start(out=bq_sb[:], in_=bq)

        qT_sb = persist.tile([128, T], bf16)  # [dk, q] all queries, q̂+bq
        kT_sb = persist.tile([128, TOWN], bf16)  # [dk, t-own]
        v8_sb = persist.tile([128, TCH, DV + 2], fp8)  # [t-part, chunk, 32v|1|0]
        pT8_sb = persist.tile([128, TCH, T], fp8)  # [t-part, chunk, q] = exp-1

        nc.vector.memset(v8_sb[:, :, DV : DV + 1], 1.0)
        nc.vector.memset(v8_sb[:, :, DV + 1 : DV + 2], 0.0)

        # x-own pairs: pair p holds chunks 2p|2p+1 side by side (4KB rows)
        xo_sb = xpool.tile([128, 3, 2 * TOWN], bf16)
        xt_sb = xpool.tile([128, CH, TOWN], bf16)
        x8o_sb = xpool.tile([128, 3, 2, TOWN], fp8)
        wqk_sb = wpool.tile([128, CH, 2 * DK], bf16)
        wv8_sb = wpool.tile([128, CH, DV], fp8)

        def xo(c):  # own-x chunk c -> [128, TOWN] slice of the pair tile
            return xo_sb[:, c // 2, (c % 2) * TOWN : (c % 2 + 1) * TOWN]

        # Input DMAs: x pairs land in q/k consumption order (pair0 sync,
        # pair1 scalar, pair2 sync). Each ring's DMAs are serialized by WAW
        # gates (the rings round-robin over all queued DMAs, so an ungated
        # late load steals bandwidth from the critical early ones). The fp8
        # copy of own-x (for the v-proj DoubleRow) is cast on-chip by the
        # vector engine, which is idle during q/k.
        nc.scalar.dma_start(out=wqk_sb[:], in_=wqk)
        nc.sync.dma_start(out=xo_sb[:, 0, :], in_=x_own[:, 0, :])
        nc.scalar.dma_start(out=xo_sb[:, 1, :], in_=x_own[:, 1, :])
        nc.sync.dma_start(out=xo_sb[:, 2, :], in_=x_own[:, 2, :])
        nc.vector.tensor_copy(wv8_sb[:, 0, 0:1], xo_sb[:, 0, 0:1])
        nc.scalar.dma_start(out=wv8_sb[:], in_=wv8)
        nc.vector.tensor_copy(xt_sb[:, 0, 0:1], xo_sb[:, 2, 0:1])
        nc.vector.tensor_copy(xt_sb[:, 3, 0:1], wv8_sb[:, 0, 1:2])
        nc.sync.dma_start(out=xt_sb[:, 0:3, :], in_=x_oth[:, 0:3, :])
        nc.scalar.dma_start(out=xt_sb[:, 3:6, :], in_=x_oth[:, 3:6, :])
        # on-chip bf16 -> fp8 casts of own-x, one per d_in chunk
        for p in range(3):
            for s in range(2):
                nc.vector.tensor_copy(
                    x8o_sb[:, p, s, :], xo_sb[:, p, s * TOWN : (s + 1) * TOWN]
                )

        def emit_scores_t(t, qh):
            # scores^T for one own-key chunk -> P'8 = fp8(exp(scale*s) - 1).
            # Two 512-col exps pipeline from two psum banks into one 1024-col
            # ph tile; a single 1024-col vector op does the -1 + fp8 cast.
            ph = ph_pool.tile([128, 1024], bf16, tag="ph")
            for n0 in (0, 512):
                ps_s = sc_pool.tile([128, 512], f32, tag="sc")
                nc.tensor.matmul(
                    ps_s[:],
                    kT_sb[:, t * 128 : (t + 1) * 128],
                    qT_sb[:, qh * 1024 + n0 : qh * 1024 + n0 + 512],
                    start=True,
                    stop=True,
                )
                nc.scalar.activation(
                    ph[:, n0 : n0 + 512],
                    ps_s[:],
                    mybir.ActivationFunctionType.Exp,
                    scale=SCALE,
                )
            nc.vector.tensor_scalar(
                out=pT8_sb[:, t, qh * 1024 : qh * 1024 + 1024],
                in0=ph[:],
                scalar1=1.0,
                scalar2=None,
                op0=mybir.AluOpType.subtract,
            )

        def emit_v_t(t):
            # v-projection for one own-key chunk: fp8 DoubleRow over d_in
            # pairs; psum accumulates 32*v; cast to fp8 keeps the 32x scale.
            ps_v = ps_pool.tile([128, 1024], f32, tag="ps")
            for p in range(3):
                for n0, n1 in ((0, 512), (512, DV)):
                    nc.tensor.matmul(
                        ps_v[:, n0:n1],
                        x8o_sb[:, p, :, t * 128 : (t + 1) * 128],
                        wv8_sb[:, 2 * p : 2 * p + 2, n0:n1],
                        start=(p == 0),
                        stop=(p == 2),
                        perf_mode=DR,
                    )
            nc.vector.tensor_copy(v8_sb[:, t, 0:DV], ps_v[:, 0:DV])

        # qc pairs share one SBUF tile and one store DMA; the last two tiles
        # store solo/split so the drain tail is short.
        o_state = {}

        def emit_out_qc(qc):
            # partial numerator + rowsum: out[qc] = sum_t P'8[t,qc].T @ [32v|1]
            # fp8 DoubleRow, t-chunk pairs in the two slots.
            ps_o = ps_pool.tile([128, 1024], f32, tag="ps")
            if qc % 2 == 0:
                o_pair = out_pool.tile([128, 2, DV + 1], fp8, tag="o")
                o_state["tile"] = o_pair
            o_sb = o_state["tile"][:, qc % 2, :]
            for n0, n1 in ((0, 512), (512, DV + 2)):
                for tp in range(4):
                    nc.tensor.matmul(
                        ps_o[:, n0:n1],
                        pT8_sb[:, 2 * tp : 2 * tp + 2, qc * 128 : (qc + 1) * 128],
                        v8_sb[:, 2 * tp : 2 * tp + 2, n0:n1],
                        start=(tp == 0),
                        stop=(tp == 3),
                        perf_mode=DR,
                    )
            for n0, n1 in ((0, 512), (512, DV + 2)):
                c1 = min(n1, DV + 1)
                # descale the 32x v8 pre-scale here: fp8 out range is +-240
                nc.vector.tensor_scalar(
                    out=o_sb[:, n0:c1],
                    in0=ps_o[:, n0:c1],
                    scalar1=1.0 / VSCALE,
                    scalar2=None,
                    op0=mybir.AluOpType.mult,
                )
                if qc >= QCH - 2:
                    # last two tiles: store each region immediately, split
                    # by partition across both rings
                    nc.sync.dma_start(
                        out=out[0:64, qc, n0:c1], in_=o_sb[0:64, n0:c1]
                    )
                    nc.scalar.dma_start(
                        out=out[64:128, qc, n0:c1], in_=o_sb[64:128, n0:c1]
                    )
                elif qc % 2 == 1 and n0 == 512:
                    # pair complete: one contiguous 2-tile store
                    eng = nc.sync if (qc // 2) % 2 == 0 else nc.scalar
                    eng.dma_start(
                        out=out[:, qc - 1 : qc + 1, :], in_=o_state["tile"][:]
                    )

        # q own-half then k own, each a single run of region-alternating mms
        # into ONE psum tile (psum switches cost a PE pipeline flush).
        # Chunk order matches DMA arrival order.
        ps_q0 = ps_pool.tile([128, 1024], f32, tag="ps")
        ps_k = ps_pool.tile([128, 1024], f32, tag="ps")
        C_ORDER = [0, 1, 2, 3, 4, 5]
        for dst, w0 in ((ps_q0, 0), (ps_k, DK)):
            for i, c in enumerate(C_ORDER):
                for n0 in (0, 512):
                    nc.tensor.matmul(
                        dst[:, n0 : n0 + 512],
                        wqk_sb[:, c, w0 : w0 + DK],
                        xo(c)[:, n0 : n0 + 512],
                        start=(i == 0),
                        stop=(i == CH - 1),
                    )
        # qT = q̂+bq on scalar, split per 512 so scores t=0 unblocks early
        for n0 in (0, 512):
            nc.scalar.activation(
                qT_sb[:, n0 : n0 + 512],
                ps_q0[:, n0 : n0 + 512],
                mybir.ActivationFunctionType.Identity,
                bias=bq_sb[:],
            )

        # scores for own queries interleaved with v-projection
        for t in range(TCH):
            nc.vector.tensor_copy(
                kT_sb[:, t * 128 : (t + 1) * 128], ps_k[:, t * 128 : (t + 1) * 128]
            )
            emit_scores_t(t, 0)
            emit_v_t(t)

        # q other-half
        ps_q1 = ps_pool.tile([128, 1024], f32, tag="ps")
        for c in range(CH):
            for n0 in (0, 512):
                nc.tensor.matmul(
                    ps_q1[:, n0 : n0 + 512],
                    wqk_sb[:, c, 0:DK],
                    xt_sb[:, c, n0 : n0 + 512],
                    start=(c == 0),
                    stop=(c == CH - 1),
                )
        for n0 in (0, 512):
            nc.scalar.activation(
                qT_sb[:, TOWN + n0 : TOWN + n0 + 512],
                ps_q1[:, n0 : n0 + 512],
                mybir.ActivationFunctionType.Identity,
                bias=bq_sb[:],
            )

        # scores for other-half queries interleaved with out
        for qc in range(8):
            emit_scores_t(qc, 1)
            emit_out_qc(qc)

        for qc in range(8, 16):
            emit_out_qc(qc)

    nc.compile()
    return nc


def _get_nc():
    if "nc" not in _CACHE:
        _CACHE["nc"] = _build()
    return _CACHE["nc"]


def _make_in_maps(x, Wq, bq, Wk, bk, Wv):
    bf16 = ml_dtypes.bfloat16
    fp8 = ml_dtypes.float8_e4m3
    wq = np.asarray(Wq, np.float32).astype(bf16).reshape(CH, 128, DK)
    wk = np.asarray(Wk, np.float32).astype(bf16).reshape(CH, 128, DK)
    base = {
        "wqk": np.ascontiguousarray(
            np.concatenate([wq, wk], axis=2).transpose(1, 0, 2)
        ),
        "wv8": np.ascontiguousarray(
            (np.asarray(Wv, np.float32) * VSCALE)
            .astype(fp8)
            .reshape(CH, 128, DV)
            .transpose(1, 0, 2)
        ),
        "bq": np.ascontiguousarray(np.asarray(bq, np.float32).reshape(DK, 1)),
    }
    in_maps = []
    for c in range(NCORES):
        b, h = c // 2, c % 2
        xb = x[b]  # [T, DIN]
        rot = np.concatenate([xb[h * TOWN :], xb[: h * TOWN]], axis=0)
        xT = rot.T.astype(bf16).reshape(CH, 128, T).transpose(1, 0, 2)  # [128,c,t]
        own = xT[:, :, 0:TOWN]  # [128, c, 1024]
        m = dict(base)
        m["x_own"] = np.ascontiguousarray(own.reshape(128, 3, 2 * TOWN))
        m["x_oth"] = np.ascontiguousarray(xT[:, :, TOWN:T])
        in_maps.append(m)
    return in_maps


def kernel(x, Wq, bq, Wk, bk, Wv, bv):
    from concourse import bass_utils

    x = np.ascontiguousarray(np.asarray(x, dtype=np.float32))
    nc = _get_nc()
    in_maps = _make_in_maps(x, Wq, bq, Wk, bk, Wv)

    res = bass_utils.run_bass_kernel_spmd(nc, in_maps, core_ids=list(range(NCORES)))

    x64 = np.asarray(x, np.float64)
    Wv64 = np.asarray(Wv, np.float64)
    bv64 = np.asarray(bv, np.float64).reshape(1, DV)
    outp = np.empty((B, T, DV), dtype=np.float32)
    for b in range(B):
        # out is partition-major [128, qc, 769] -> [qc*128+p, 769]
        p0 = res.results[2 * b]["out"].transpose(1, 0, 2).reshape(T, DV + 1)
        p1 = res.results[2 * b + 1]["out"].transpose(1, 0, 2).reshape(T, DV + 1)
        p1 = np.concatenate([p1[TOWN:], p1[:TOWN]], axis=0)
        s = p0.astype(np.float64) + p1.astype(np.float64)
        colsum = x64[b].sum(axis=0) @ Wv64  # exact f64 restore of the
        num = s[:, 0:DV] + colsum[None, :]  # dropped "+1" in expm1
        den = s[:, DV : DV + 1] * VSCALE + float(T)
        outp[b] = (num / den + bv64).astype(np.float32)
    return outp
